# revision 47
# baseline (speedup 1.0000x reference)
"""Trainium2 Bass kernel for nn_Block_19121194402322 (dense_transformer).

Math notes (validated numerically against the reference):
  - The reference einsum 'bnqk,bnvd->bnqd' contracts over BOTH k and v, so
    out[b,n,q,d] = (sum_k softmax(...)[q,k]) * (sum_v v[b,n,v,d]).  Softmax rows
    sum to exactly 1, so the whole Q/K/softmax pipeline is dead code; the
    attention output is the per-head sum of v broadcast over q.
  - After the (non-standard) reshape, head n owns flat sub-rows
    r in [1024n, 1024(n+1)) of (x@Wv).reshape(12288, 64), r = 12 s + c.
    With a 0/1 selector A (rows indexed by (c,n)):  Y = A @ x, and
    wT[d,n] = sum_{c,k} (Wv block)^T (Y^T col) -- 72 matmuls of 12 cols.
  - a = x + LN1(w) adds a per-batch CONSTANT vector, so the MLP's first
    matmul splits:  W1^T a^T = W1^T x^T + (W1^T lnvec) 1^T.  H^T = W1^T x^T
    is computed directly and u1 = W1^T lnvec folds into the gelu bias.
  - MLP: g = gelu(W1^T x^T + b1 + u1); m = gelu(g^T W2 + b2); out = x + LN2(m).

fp8 strategy (e4m3, PE DoubleRow perf mode = 0.5 cycles/output-col per
K=256 pair -- 2x the bf16 rate per product):
  - W1, W2 scaled by 2^10 and split exactly on host: Wh = fp8(W*1024),
    Wl = fp8(W*1024 - Wh).  The 2^-10 rescale folds into the gelu
    activation scale.
  - stage5 (2p): x quantized once to fp8 on host (xth); product
    (W1h+W1l)^T xth via DR pairs.  Final rel-err ~1.6e-2 (budget 2e-2).
    S5_3P adds the W1h*xtl correction (rel-err ~3e-3) at +18k PE cycles.
  - stage6 (3p): g kept to ~bf16 accuracy as gh=fp8(g_bf) (DVE cast) +
    gl=fp8(g_bf-gh) (Pool sub); products gh*W2h + gh*W2l + gl*W2h.
  - stage1/stage2/LN paths stay bf16 (tiny PE cost).

Distribution: pure data-parallel over batch B=8 across 8 NeuronCores
(one batch element per core); weights replicated; no collectives.
"""

import numpy as np

S = 1024
E = 768
HID = 1536
HEADS = 12
HD = 64
EPS = 1e-5
P = 128
N_CORES = 8
ACOLS = HEADS * HEADS  # 144 selector columns: col = n*12 + c
KE = E // P    # 6
KH = HID // P  # 12
OT = S // P    # 8 token tiles
SW = 1024.0    # pow2 scale for W1/W2 fp8 split
ISW = 1.0 / SW
WARMUP_MM = 40
S5_3P = False  # stage5: add W1h*xtl correction products

_CACHE = {}


def _build_selector():
    """Selector, active columns only: each 128-token chunk o touches exactly
    heads nlo(o) and nlo(o)+1 (nlo = 3o//2), so only 24 of the 144 (n,c)
    columns are nonzero per chunk.  at2[p, o, j] =
    [head(o*128+p, c=j%12) == nlo(o) + j//12]."""
    at2 = np.zeros((P, OT, 24), np.float32)
    for o in range(OT):
        nlo = (3 * o) // 2
        for p in range(P):
            s = o * P + p
            for c in range(HEADS):
                n = (HEADS * s + c) // S
                j = (n - nlo) * HEADS + c
                at2[p, o, j] = 1.0
    return at2


def _split_multi_waits(m):
    """Hoist all-but-one sync waits of each instruction onto preceding
    single-wait EventSemaphore instructions on the same engine.  Several TPB
    instruction structs carry only one sync-wait slot, and walrus codegen
    errors on more."""
    counter = [0]

    def fix_block(blk):
        out = []
        for inst in blk.get("instructions", []):
            si = inst.get("sync_info")
            waits = (si or {}).get("on_wait") or []
            if si and len(waits) > 1 and inst.get("opcode") != "EventSemaphore":
                for w in waits[:-1]:
                    counter[0] += 1
                    out.append({
                        "debug": inst.get("debug", 0), "engine": inst["engine"],
                        "ins": [], "outs": [], "name": f"I-wsplit-{counter[0]}",
                        "opcode": "EventSemaphore",
                        "sync_info": {"on_update": [], "on_wait": [w]},
                    })
                si["on_wait"] = waits[-1:]
            out.append(inst)
        blk["instructions"] = out
        for sub in blk.get("blocks", []):
            fix_block(sub)

    for fn in m["functions"]:
        for blk in fn["blocks"]:
            fix_block(blk)
    return m


def _build_bass(flags):
    import json
    import concourse.bass as bass
    import concourse.mybir as mybir
    import concourse.tile as tile

    g1_ones, be1_zero, b1_zero, g2_ones, be2_zero, b2_zero = flags
    general6 = not (g2_ones and be2_zero and b2_zero)

    f32 = mybir.dt.float32
    bf16 = mybir.dt.bfloat16
    fp8 = mybir.dt.float8e4
    AX = mybir.AxisListType.X
    OP = mybir.AluOpType
    AF = mybir.ActivationFunctionType
    DR = mybir.MatmulPerfMode.DoubleRow

    nc = bass.Bass(trn_type="TRN2")

    # ---- DRAM parameters (host pre-packs layouts; see _pack_inputs) ------
    xb_d = nc.declare_dram_parameter("xb", [S, E], bf16, isOutput=False)
    atsel_d = nc.declare_dram_parameter("atsel", [P, OT * 24], bf16,
                                        isOutput=False)
    wv_d = nc.declare_dram_parameter("wv", [E, E], bf16, isOutput=False)
    xth_d = nc.declare_dram_parameter("xth", [P, KE * S], fp8, isOutput=False)
    if S5_3P:
        xtl_d = nc.declare_dram_parameter("xtl", [P, KE * S], fp8,
                                          isOutput=False)
    w1_d = nc.declare_dram_parameter("w1", [P, KH * 2 * KE * P], fp8,
                                     isOutput=False)
    w2h_d = nc.declare_dram_parameter("w2h", [HID, E], fp8, isOutput=False)
    w2l_d = nc.declare_dram_parameter("w2l", [HID, E], fp8, isOutput=False)
    svec_d = nc.declare_dram_parameter("svec", [P, 64], f32, isOutput=False)
    if general6:
        bcast_d = nc.declare_dram_parameter("bcast", [P, 3 * E], bf16,
                                            isOutput=False)
    out_d = nc.declare_dram_parameter("out", [S, E], bf16, isOutput=True)

    H2 = E // 2  # 384
    x_v = xb_d[:].rearrange("(o p) f -> p o f", p=P)    # (128, 8, 768)
    out_v = out_d[:].rearrange("(o p) f -> p o f", p=P)
    wv_v = wv_d[:].rearrange("(k p) f -> p k f", p=P)   # (128, 6, 768)
    w2h_v = w2h_d[:].rearrange("(k p) f -> p k f", p=P)  # (128, 12, 768)
    w2l_v = w2l_d[:].rearrange("(k p) f -> p k f", p=P)

    with tile.TileContext(nc) as tc:
        with (
            tc.tile_pool(name="xbp", bufs=1) as xbp,
            tc.tile_pool(name="atp", bufs=1) as atp,
            tc.tile_pool(name="wvp", bufs=1) as wvp,
            tc.tile_pool(name="xtp", bufs=1) as xtp,
            tc.tile_pool(name="w1p", bufs=1) as w1p,
            tc.tile_pool(name="w2p", bufs=1) as w2p,
            tc.tile_pool(name="gp", bufs=1) as gp,
            tc.tile_pool(name="ytp", bufs=1) as ytp,
            tc.tile_pool(name="cst", bufs=1) as cst,
            tc.tile_pool(name="sm", bufs=1) as sm,
            tc.tile_pool(name="gbp", bufs=3) as gbp,
            tc.tile_pool(name="mp", bufs=3) as mp,
            tc.tile_pool(name="upl", bufs=3) as upl,
            tc.tile_pool(name="stp", bufs=3) as stp,
            tc.tile_pool(name="psB", bufs=5, space="PSUM") as psB,
            tc.tile_pool(name="psT", bufs=2, space="PSUM") as psT,
            tc.tile_pool(name="psS", bufs=1, space="PSUM") as psS,
        ):
            # ---- DMA loads: ordering = arrival pacing (transfers serialize
            # on one ~360GB/s resource) ---------------------------------------
            atsel_sb = atp.tile([P, OT * 24], bf16)
            nc.sync.dma_start(out=atsel_sb, in_=atsel_d[:])

            xb_sb = xbp.tile([P, OT, E], bf16)
            for c in range(4):
                nc.sync.dma_start(out=xb_sb[:, 2 * c:2 * c + 2, :],
                                  in_=x_v[:, 2 * c:2 * c + 2, :])

            svec = cst.tile([P, 64], f32)
            nc.gpsimd.dma_start(out=svec, in_=svec_d[:])
            b1col = svec[:, 0:12]
            g1col = svec[:, 12:18]
            be1col = svec[:, 18:24]

            wv_sb = wvp.tile([P, KE, E], bf16)
            nc.sync.dma_start(out=wv_sb, in_=wv_v)

            xth_sb = xtp.tile([P, KE, S], fp8)
            xth_v = xth_d[:].rearrange("p (k s) -> p k s", k=KE)
            nc.sync.dma_start(out=xth_sb[:, :, 0:512], in_=xth_v[:, :, 0:512])

            # W1 hi+lo in j2-pair blocks: big enough that the per-DMA 900ns
            # completion latency amortizes ahead of stage5's consumption
            w1_sb = w1p.tile([P, KH, 2, KE, P], fp8)
            w1_v = w1_d[:]
            BLK = 2 * 2 * KE * P  # one j2-pair
            for jp in range(6):
                nc.sync.dma_start(
                    out=w1_sb[:, 2 * jp:2 * jp + 2].rearrange(
                        "p j h k c -> p (j h k c)"),
                    in_=w1_v[:, jp * BLK:(jp + 1) * BLK])
            nc.sync.dma_start(out=xth_sb[:, :, 512:1024],
                              in_=xth_v[:, :, 512:1024])
            if S5_3P:
                xtl_sb = xtp.tile([P, KE, S], fp8)
                nc.sync.dma_start(
                    out=xtl_sb, in_=xtl_d[:].rearrange("p (k s) -> p k s",
                                                       k=KE))

            w2l_sb = w2p.tile([P, KH, E], fp8)
            nc.sync.dma_start(out=w2l_sb, in_=w2l_v)
            w2h_sb = w2p.tile([P, KH, E], fp8)
            nc.sync.dma_start(out=w2h_sb[:, 0:6], in_=w2h_v[:, 0:6])
            nc.sync.dma_start(out=w2h_sb[:, 6:12], in_=w2h_v[:, 6:12])
            if general6:
                bcast = cst.tile([P, 3 * E], bf16)
                nc.scalar.dma_start(out=bcast, in_=bcast_d[:])
                b2row = bcast[:, 0:E]      # SW * b2 (row 0 used)
                g2b = bcast[:, E:2 * E]
                be2b = bcast[:, 2 * E:3 * E]

            # ---- small constants --------------------------------------------
            jsrc = sm.tile([P, P], bf16)
            nc.vector.memset(jsrc, 0.0)
            eps_sb = sm.tile([P, 1], f32)
            nc.vector.memset(eps_sb, EPS)
            ones64 = sm.tile([64, 1], bf16)
            nc.vector.memset(ones64, 1.0)
            if general6 and not b2_zero:
                ones1 = sm.tile([1, P], bf16)
                nc.vector.memset(ones1, 1.0)

            # ---- PE warm-up: burn the p-state ramp during the DMA window ----
            psj = psS.tile([P, 512], f32, tag="s")
            for _ in range(WARMUP_MM):
                nc.tensor.matmul(psj[:, 0:P], jsrc, jsrc,
                                 start=True, stop=True)

            def dummies(n):
                for _ in range(n):
                    nc.tensor.matmul(psj[:, 0:P], jsrc, jsrc,
                                     start=True, stop=True)

            # ---- stage 1: Y^T accumulation (paced by xb arrival) ------------
            # yt[p(e), k, n*12+c] = sum_s x[s, k*128+p] * A[s, (n,c)]
            def emit_s1(ps, i, o):
                nlo = (3 * o) // 2
                for half in range(2):
                    gc = (nlo + half) * HEADS
                    rel = half * HEADS
                    first = not (o % 2 == 1 and half == 0)
                    last = not (o % 2 == 0 and half == 1)
                    nc.tensor.matmul(
                        ps[:, gc:gc + HEADS],
                        xb_sb[:, o, i * P:(i + 1) * P],
                        atsel_sb[:, o * 24 + rel:o * 24 + rel + HEADS],
                        start=first, stop=last,
                    )

            yt_sb = ytp.tile([P, KE, ACOLS], bf16)
            s1ps = [psB.tile([P, 512], f32, tag="big", name=f"s1ps{i}")
                    for i in range(3)]
            for o in range(OT):
                for i in range(3):
                    emit_s1(s1ps[i], i, o)
                if o < OT - 1:
                    dummies(6)
            for i in range(3):
                nc.vector.tensor_copy(yt_sb[:, i, :], s1ps[i][:, :ACOLS])
            for i in range(3, KE):
                ps = psB.tile([P, 512], f32, tag="big")
                for o in range(OT):
                    emit_s1(ps, i, o)
                nc.vector.tensor_copy(yt_sb[:, i, :], ps[:, :ACOLS])
            # keep the p-state ramp hot while stage2 waits for the wv DMA
            dummies(52)

            # ---- stage 2 (transposed): wT[d, n] in PSUM ---------------------
            # wT[d,n] = sum_{k,c} Wv[:,k,c*64+d]^T . ytcol(k, n*12+c)
            ps_wT = psT.tile([P, 512], f32, tag="t")
            n_mm = 0
            for k in range(KE):
                ytk = yt_sb[:, k, :].rearrange("p (n c) -> p c n", c=HEADS)
                for c in range(HEADS):
                    n_mm += 1
                    nc.tensor.matmul(
                        ps_wT[:HD, :HEADS],
                        wv_sb[:, k, c * HD:(c + 1) * HD],
                        ytk[:, c, :],
                        start=(n_mm == 1), stop=(n_mm == KE * HEADS),
                    )
            # cover the xth1/w1 DMA wait before stage5 so the p-state ramp
            # stays hot going into the big GEMM streams
            dummies(36)

            # ---- stage 5 setup ----------------------------------------------
            g_bf_t = {}  # (j2, half) -> bf16 gelu tile
            g_h = gp.tile([P, KH, S], fp8)
            g_l = gp.tile([P, KH, S], fp8)
            biascol = sm.tile([P, KH], f32)

            def s5_mm(j2, half, ps):
                cs = half * 512
                first = True
                for hl in range(2):
                    for g in range(3):
                        nc.tensor.matmul(
                            ps, w1_sb[:, j2, hl, 2 * g:2 * g + 2, :],
                            xth_sb[:, 2 * g:2 * g + 2, cs:cs + 512],
                            start=first,
                            stop=(not S5_3P and hl == 1 and g == 2),
                            perf_mode=DR)
                        first = False
                if S5_3P:
                    for g in range(3):
                        nc.tensor.matmul(
                            ps, w1_sb[:, j2, 0, 2 * g:2 * g + 2, :],
                            xtl_sb[:, 2 * g:2 * g + 2, cs:cs + 512],
                            start=False, stop=(g == 2), perf_mode=DR)

            def s5_post(j2, half, ps):
                # ACT: g_bf = gelu(psum * 2^-10 + biascol); gh cast mostly on
                # Pool (806ns) with a few tiles' casts on ACT (which has
                # ~180ns/tile slack vs Pool) to shorten the h0 chain that
                # gates stage6 chunk 0; DVE: gl = g_bf - gh (643ns).
                bias_j2(j2)
                cs = half * 512
                gbt = gbp.tile([P, 512], bf16, tag="gb")
                nc.scalar.activation(out=gbt, in_=ps, func=AF.Gelu,
                                     bias=biascol[:, j2:j2 + 1], scale=ISW)
                if half == 0 and j2 in (5, 11):
                    nc.scalar.activation(out=g_h[:, j2, cs:cs + 512],
                                         in_=gbt, func=AF.Copy)
                else:
                    nc.gpsimd.tensor_copy(g_h[:, j2, cs:cs + 512], gbt)
                nc.vector.tensor_sub(g_l[:, j2, cs:cs + 512], gbt,
                                     g_h[:, j2, cs:cs + 512])

            # ---- LN1 chain (emitted after stage5 j2=0,1 so PE stays busy) ---
            def ln1_chain():
                # wsq = [wT | wT^2] bf16; column sums via PE ones-matmul
                wsq = sm.tile([64, 24], bf16)
                nc.scalar.activation(out=wsq[:, 0:12], in_=ps_wT[:HD, :HEADS],
                                     func=AF.Copy)
                nc.vector.tensor_mul(wsq[:, 12:24], wsq[:, 0:12],
                                     wsq[:, 0:12])
                ps_s = psT.tile([P, 512], f32, tag="t")
                nc.tensor.matmul(ps_s[:1, 0:24], ones64, wsq,
                                 start=True, stop=True)
                tots = sm.tile([1, 2], f32)
                nc.vector.tensor_reduce(
                    out=tots,
                    in_=ps_s[:1, 0:24].rearrange("p (a c) -> p a c", a=2),
                    axis=AX, op=OP.add)
                nc.vector.tensor_scalar_mul(tots, tots, 1.0 / E)
                mu2 = sm.tile([1, 1], f32)
                nc.vector.tensor_mul(mu2, tots[:, 0:1], tots[:, 0:1])
                mr = sm.tile([32, 2], f32)
                nc.vector.memset(mr, 0.0)
                nc.vector.tensor_sub(mr[:1, 1:2], tots[:, 1:2], mu2)
                nc.scalar.activation(out=mr[:1, 1:2], in_=mr[:1, 1:2],
                                     func=AF.Sqrt, bias=eps_sb[:1])
                nc.vector.reciprocal(mr[:1, 1:2], mr[:1, 1:2])
                nc.vector.tensor_copy(mr[:1, 0:1], tots[:, 0:1])
                mrb = sm.tile([P, 2], f32)
                for q in range(4):
                    nc.vector.stream_shuffle(mrb[32 * q:32 * (q + 1), :],
                                             mr[:, :], [0] * 32)
                # wcol[p, k] = w[k*128+p]: flat e = n*64+d ->
                #   p<64:  wT[p, 2k];  p>=64: wT[p-64, 2k+1]
                wcol = sm.tile([P, KE], f32)
                wTv = ps_wT[:HD, :HEADS].rearrange("p (k two) -> p k two",
                                                   two=2)
                nc.vector.tensor_copy(wcol[0:64, :], wTv[:, :, 0])
                nc.vector.tensor_copy(wcol[64:128, :], wTv[:, :, 1])
                lnc = sm.tile([P, KE], f32)
                nc.vector.tensor_scalar(lnc, wcol, mrb[:, 0:1], mrb[:, 1:2],
                                        OP.subtract, OP.mult)
                if not g1_ones:
                    nc.vector.tensor_mul(lnc, lnc, g1col)
                if not be1_zero:
                    nc.vector.tensor_add(lnc, lnc, be1col)
                lnh = sm.tile([P, KE], fp8)
                nc.vector.tensor_copy(lnh, lnc)
                lnl = sm.tile([P, KE], fp8)
                nc.vector.tensor_sub(lnl, lnc, lnh)
                return lnh, lnl

            pu_tiles = {}
            lnq_ref = []

            def u1_j2(j2):
                # u1[:, j2] = ((W1h+W1l)^T lnh + W1h^T lnl)[j2 block].
                # One psum tile per j2 (rotating through psT): dependency
                # tracking is tile-granular, so gelu(j2) waits only w1[j2].
                # Lives in psT (NOT psS) so the dummy bank psj isn't aliased.
                lnh, lnl = lnq_ref[0]
                pu = psT.tile([P, 512], f32, tag="t")
                pu_tiles[j2] = pu
                terms = [(0, lnh), (1, lnh), (0, lnl)]
                n = 0
                for hl, ln in terms:
                    for k in range(KE):
                        n += 1
                        nc.tensor.matmul(
                            pu[:, j2:j2 + 1],
                            w1_sb[:, j2, hl, k, :],
                            ln[:, k:k + 1],
                            start=(n == 1), stop=(n == 3 * KE))

            bias_done = [False] * KH

            def bias_j2(j2, _unused=None):
                if bias_done[j2]:
                    return
                bias_done[j2] = True
                nc.scalar.activation(out=biascol[:, j2:j2 + 1],
                                     in_=pu_tiles[j2][:, j2:j2 + 1],
                                     func=AF.Copy, scale=ISW)
                if not b1_zero:
                    nc.vector.tensor_add(biascol[:, j2:j2 + 1],
                                         biascol[:, j2:j2 + 1],
                                         b1col[:, j2:j2 + 1])

            # NOTE: biascol folds u1 with the 2^-10 PSUM scale: the gelu does
            # gelu(ps*ISW + bias) and ps holds 1024*(W1^T x), u1 PSUM holds
            # 1024*u1, so bias = pu*ISW + b1 exactly.

            # ---- stage 5 main loop ------------------------------------------
            s5_live = []

            def s5_tile(j2, half, pool=None):
                ps = (pool or psB).tile([P, 512], f32,
                                        tag=("t" if pool else "big"))
                s5_mm(j2, half, ps)
                s5_live.append((j2, half, ps))

            def s5_drain(n_keep=0):
                while len(s5_live) > n_keep:
                    s5_post(*s5_live.pop(0))

            s5_tile(0, 0)
            lnq_ref.append(ln1_chain())
            u1_j2(0)
            u1_j2(1)
            s5_tile(1, 0)
            for j2 in range(2, KH):
                u1_j2(j2)
                s5_tile(j2, 0)
                s5_drain(2)
            for j2 in range(0, 4):
                s5_tile(j2, 1)
                s5_drain(2)

            # ---- stage 6: m = gelu((g^T W2)*ISW + b2); out = x + LN2(m) -----
            LAST = OT - 1

            if general6 and not be2_zero:
                xrt = {}
                for o in range(OT):
                    xr = stp.tile([P, E], bf16, tag="xr")
                    nc.gpsimd.tensor_add(xr, xb_sb[:, o, :], be2b)
                    xrt[o] = xr

            def s6_pieces(o):
                return ([(0, 256), (256, 512), (512, E)] if o == LAST
                        else [(0, H2), (H2, E)])

            def s6_piece_mms(o, cs, ce, ps):
                # W2l products first (w2h is the last weight DMA to land);
                # within each product, k-pairs ascending (w2h arrives in two
                # k-halves)
                n = 0
                for gt, wt in ((g_h, w2l_sb), (g_h, w2h_sb), (g_l, w2h_sb)):
                    for g2 in range(KE):
                        n += 1
                        nc.tensor.matmul(
                            ps[:, :ce - cs],
                            gt[:, 2 * g2:2 * g2 + 2, o * P:(o + 1) * P],
                            wt[:, 2 * g2:2 * g2 + 2, cs:ce],
                            start=(n == 1),
                            stop=(n == 3 * KE
                                  and (not general6 or b2_zero)),
                            perf_mode=DR)
                if general6 and not b2_zero:
                    nc.tensor.matmul(ps[:, :ce - cs], ones1,
                                     b2row[0:1, cs:ce],
                                     start=False, stop=True)

            def s6_piece_post(o, h, cs, ce, ps, m_bf, stats):
                nc.scalar.activation(out=m_bf[:, cs:ce], in_=ps[:, :ce - cs],
                                     func=AF.Gelu, scale=ISW)
                nc.vector.bn_stats(out=stats[:, h, :], in_=m_bf[:, cs:ce])

            def s6_early(o):
                pieces = s6_pieces(o)
                m_bf = mp.tile([P, E], bf16, tag="m")
                stats = stp.tile([P, 3, 6], f32, tag="st")
                if o == 0:
                    # interleave both piece groups so the w2l-only work of
                    # both pieces runs while the w2h DMA finishes
                    pss = []
                    for h, (cs, ce) in enumerate(pieces):
                        ps = psB.tile([P, 512], f32, tag="big")
                        pss.append(ps)
                        n = 0
                        for g2 in range(KE):
                            n += 1
                            nc.tensor.matmul(
                                ps[:, :ce - cs],
                                g_h[:, 2 * g2:2 * g2 + 2, 0:P],
                                w2l_sb[:, 2 * g2:2 * g2 + 2, cs:ce],
                                start=(n == 1), stop=False, perf_mode=DR)
                    for h, (cs, ce) in enumerate(pieces):
                        ps = pss[h]
                        n = 0
                        for gt in (g_h, g_l):
                            for g2 in range(KE):
                                n += 1
                                nc.tensor.matmul(
                                    ps[:, :ce - cs],
                                    gt[:, 2 * g2:2 * g2 + 2, 0:P],
                                    w2h_sb[:, 2 * g2:2 * g2 + 2, cs:ce],
                                    start=False,
                                    stop=(n == 2 * KE
                                          and (not general6 or b2_zero)),
                                    perf_mode=DR)
                        if general6 and not b2_zero:
                            nc.tensor.matmul(ps[:, :ce - cs], ones1,
                                             b2row[0:1, cs:ce],
                                             start=False, stop=True)
                        s6_piece_post(o, h, cs, ce, ps, m_bf, stats)
                    return m_bf, stats
                for h, (cs, ce) in enumerate(pieces):
                    ps = psB.tile([P, 512], f32, tag="big")
                    s6_piece_mms(o, cs, ce, ps)
                    s6_piece_post(o, h, cs, ce, ps, m_bf, stats)
                return m_bf, stats

            def s6_late(o, m_bf, stats):
                # last chunk: process the final piece first so its (short)
                # sync store path starts early, and put the wide merged
                # store on gpsimd SWDGE -- the two DGE paths then complete
                # in parallel at ~the same time
                pieces = s6_pieces(o)
                order = [2, 0, 1] if o == LAST else range(len(pieces))
                mv = stp.tile([P, 2], f32, tag="mv")
                nc.vector.bn_aggr(out=mv, in_=stats[:, 0:len(pieces), :])
                std = stp.tile([P, 1], f32, tag="std")
                nc.scalar.activation(out=std, in_=mv[:, 1:2], func=AF.Sqrt,
                                     bias=eps_sb)
                nc.vector.reciprocal(std, std)
                u_bf = upl.tile([P, E], bf16, tag="u")
                for h in order:
                    cs, ce = pieces[h]
                    nc.vector.tensor_scalar(u_bf[:, cs:ce], m_bf[:, cs:ce],
                                            mv[:, 0:1], std,
                                            OP.subtract, OP.mult)
                    if general6 and not g2_ones:
                        nc.vector.tensor_mul(u_bf[:, cs:ce], u_bf[:, cs:ce],
                                             g2b[:, cs:ce])
                    res = (xrt[o][:, cs:ce] if general6 and not be2_zero
                           else xb_sb[:, o, cs:ce])
                    nc.vector.tensor_add(u_bf[:, cs:ce], u_bf[:, cs:ce], res)
                    # last chunk: one merged sync store for pieces 0+1, and
                    # the final piece via gpsimd SWDGE (Pool idle) so the two
                    # paths complete in parallel instead of serializing on
                    # HWDGE; other chunks store per piece on sync.
                    if o == LAST:
                        if h == 2:
                            nc.gpsimd.dma_start(out=out_v[:, o, cs:ce],
                                                in_=u_bf[:, cs:ce])
                        elif h == 1:
                            nc.sync.dma_start(out=out_v[:, o, 0:512],
                                              in_=u_bf[:, 0:512])
                    else:
                        nc.sync.dma_start(out=out_v[:, o, cs:ce],
                                          in_=u_bf[:, cs:ce])

            # stage6 chunk o consumes only token block o of g, so chunks
            # 0..3 need just the first stage5 token-half; the rest of the
            # half-1 tiles are emitted between chunks 0 and 1 so the gh/gl
            # chain drains while the PE works on stage6.
            prev = None
            for o in range(OT):
                if o == 1:
                    for j2 in range(4, 8):
                        s5_tile(j2, 1)
                        s5_drain(2)
                elif o == 2:
                    for j2 in range(8, KH):
                        s5_tile(j2, 1)
                        s5_drain(2)
                    s5_drain(0)
                if o == LAST and prev is not None:
                    s6_late(o - 1, *prev)
                    prev = None
                cur = s6_early(o)
                if prev is not None:
                    s6_late(o - 1, *prev)
                prev = cur
            s6_late(OT - 1, *prev)

    m = json.loads(mybir.module_to_json_bytes(nc.m))
    m = _split_multi_waits(m)
    nc.m = mybir.module_from_json_bytes(json.dumps(m).encode())
    return nc


def _get_nc(flags):
    key = ("nc", flags)
    if key not in _CACHE:
        _CACHE[key] = _build_bass(flags)
    return _CACHE[key]


def _pack_inputs(inputs):
    import ml_dtypes
    bf = ml_dtypes.bfloat16
    f8 = ml_dtypes.float8_e4m3

    def c(a, dt=bf):
        return np.ascontiguousarray(np.asarray(a), dtype=dt)

    def split8(W):
        Ws = np.clip(np.asarray(W, np.float32) * SW, -240.0, 240.0)
        Wh = Ws.astype(f8)
        Wl = np.clip(Ws - Wh.astype(np.float32), -240.0, 240.0).astype(f8)
        return Wh, Wl

    at2 = _build_selector()  # (128, OT, 24)
    atsel = at2.reshape(P, OT * 24)

    W1h, W1l = split8(inputs["W1"])  # (768, 1536)

    def w1pack(W):  # -> [p, j2, k, c] flat
        return (W.reshape(KE, P, KH, P).transpose(1, 2, 0, 3)
                .reshape(P, KH, KE * P))

    w1 = np.stack([w1pack(W1h), w1pack(W1l)], axis=2)  # [p, j2, 2, k*c]
    w1 = w1.reshape(P, KH * 2 * KE * P)

    W2h, W2l = split8(inputs["W2"])  # (1536, 768)

    svec = np.zeros((P, 64), np.float32)
    svec[:, 0:12] = np.asarray(inputs["b1"], np.float32).reshape(KH, P).T
    svec[:, 12:18] = np.asarray(inputs["g1"], np.float32).reshape(KE, P).T
    svec[:, 18:24] = np.asarray(inputs["beta1"], np.float32).reshape(KE, P).T

    bcast = np.concatenate([
        np.asarray(inputs["b2"], np.float32) * SW,
        np.asarray(inputs["g2"], np.float32),
        np.asarray(inputs["beta2"], np.float32),
    ])[None, :].repeat(P, axis=0)

    shared = {
        "atsel": c(atsel),
        "wv": c(inputs["Wv"]),
        "w1": np.ascontiguousarray(w1),
        "w2h": np.ascontiguousarray(W2h),
        "w2l": np.ascontiguousarray(W2l),
        "svec": c(svec, np.float32),
    }
    flags = (
        bool(np.all(np.asarray(inputs["g1"]) == 1.0)),
        bool(np.all(np.asarray(inputs["beta1"]) == 0.0)),
        bool(np.all(np.asarray(inputs["b1"]) == 0.0)),
        bool(np.all(np.asarray(inputs["g2"]) == 1.0)),
        bool(np.all(np.asarray(inputs["beta2"]) == 0.0)),
        bool(np.all(np.asarray(inputs["b2"]) == 0.0)),
    )
    general6 = not (flags[3] and flags[4] and flags[5])
    if general6:
        shared["bcast"] = c(bcast)

    x = np.asarray(inputs["x"], np.float32)
    maps = []
    for b in range(N_CORES):
        xb = x[b]
        xh = xb.astype(f8)                       # (S, E)
        xth = np.ascontiguousarray(
            xh.T.reshape(KE, P, S).transpose(1, 0, 2).reshape(P, KE * S))
        imap = dict(shared, xb=c(xb), xth=xth)
        if S5_3P:
            xl = (xb - xh.astype(np.float32)).astype(f8)
            imap["xtl"] = np.ascontiguousarray(
                xl.T.reshape(KE, P, S).transpose(1, 0, 2).reshape(P, KE * S))
        maps.append(imap)
    return maps, flags


def _run(inputs, trace=False):
    from concourse.bass_utils import run_bass_kernel_spmd

    in_maps, flags = _pack_inputs(inputs)
    nc = _get_nc(flags)
    _CACHE["last_nc"] = nc
    res = run_bass_kernel_spmd(
        nc, in_maps, core_ids=list(range(N_CORES)), trace=trace,
        **({"trace_cores": list(range(N_CORES))} if trace else {}),
    )
    out = np.stack(
        [np.asarray(r["out"], dtype=np.float32) for r in res.results], axis=0)
    return out, res


def kernel(x, Wq=None, Wk=None, Wv=None, W1=None, b1=None, W2=None, b2=None,
           g1=None, beta1=None, g2=None, beta2=None):
    out, _ = _run(dict(x=x, Wv=Wv, W1=W1, b1=b1, W2=W2, b2=b2, g1=g1,
                       beta1=beta1, g2=g2, beta2=beta2))
    return out


def kernel_profiled(**inputs):
    out, res = _run(inputs, trace=True)
    return out, res
